# revision 1
# baseline (speedup 1.0000x reference)
"""AGCN (adaptive graph conv) distributed Bass kernel for 8 TRN2 NeuronCores.

Sharding: data-parallel over batch B=32 -> 4 batches/core, no collectives.
Per core:
  E^T[m,n] = max(exp(nv1 . nv2), 1)      (== exp(relu(z)), built transposed)
  U1 = E^T' @ [X | 1]  -> Y1 = U1[:, :C] / d   (d = row sums via ones column)
  U2 = E^T' @ Y1       -> Y2 = 2*U2/d - X
  out[b,n,o] = sum_d emb[n,d] * (x_g[b,n,:] @ Wf[:, (o,d)]) + bias[n,o]
               with x_g = [X, Y1, Y2] (Chebyshev K=3), via Z-form matmuls.
Matmul inputs bf16 (z-matmul in float32r), PSUM accumulation fp32.

Schedule: adjacency chunks software-pipelined 1:4 with hop1 matmuls so the
PE never waits on the scalar-engine exp pipe; hop2 + PE-transposes + the
combine stage are fused per node tile with the combine lagging one tile.
The PSUM-Z drain work is spread over three engine paths (V: DVE-direct,
A: ACT drain + DVE tree-reduce, G: ACT drain + GPSIMD tree-reduce) in the
ratio 8:24:32 picked by TimelineSim measurement.
"""

import sys

for _p in ("/opt/trn_rl_repo",):
    if _p not in sys.path:
        sys.path.insert(0, _p)

from contextlib import ExitStack

import ml_dtypes
import numpy as np

import concourse.bass as bass  # noqa: F401  (bass import keeps mybir registry happy)
import concourse.tile as tile
from concourse import bacc, mybir
from concourse.bass_utils import run_bass_kernel_spmd

BF16 = ml_dtypes.bfloat16

B, N, DIN, DOUT, EMB, CHEB = 32, 2000, 64, 64, 16, 3
CORES = 8
BLOC = B // CORES          # 4 batches per core
CFREE = BLOC * DIN         # 256
CAUG = CFREE + 1           # 257 (ones column for row sums)
P = 128
NT = (N + P - 1) // P      # 16 node tiles (last = 80 rows)
KI = CHEB * DIN            # 192 contraction (k,i)
DO = EMB * DOUT            # 1024 (d,o) free


def _tsz(t: int) -> int:
    return min(P, N - t * P)


def _build():
    nc = bacc.Bacc("TRN2", target_bir_lowering=False, debug=False)
    f32, bf16, f32r = mybir.dt.float32, mybir.dt.bfloat16, mybir.dt.float32r
    AF = mybir.ActivationFunctionType
    OP = mybir.AluOpType

    xaug = nc.declare_dram_parameter("xaug", [N, CAUG], bf16, isOutput=False)
    nv2 = nc.declare_dram_parameter("nv2", [EMB, N], f32r, isOutput=False)
    nv1t = nc.declare_dram_parameter("nv1t", [EMB, N], f32r, isOutput=False)
    wf2 = nc.declare_dram_parameter("wf2", [2, P, DO], bf16, isOutput=False)
    eexp = nc.declare_dram_parameter("eexp", [N, DOUT, EMB], bf16, isOutput=False)
    biasf = nc.declare_dram_parameter("biasf", [N, DOUT], f32, isOutput=False)
    outp = nc.declare_dram_parameter("out", [BLOC, N, DOUT], f32, isOutput=True)

    with tile.TileContext(nc) as tc, ExitStack() as ctx:
        sing = ctx.enter_context(tc.tile_pool(name="sing", bufs=1))
        wrk = ctx.enter_context(tc.tile_pool(name="wrk", bufs=5))
        wrk2 = ctx.enter_context(tc.tile_pool(name="wrk2", bufs=3))
        pz_ = ctx.enter_context(tc.tile_pool(name="pz", bufs=2, space="PSUM"))
        pu_ = ctx.enter_context(tc.tile_pool(name="pu", bufs=2, space="PSUM"))
        pZ_ = ctx.enter_context(tc.tile_pool(name="pZ", bufs=2, space="PSUM"))

        NPAD = NT * P  # 2048: transpose targets padded so 80-row tiles can't spill

        # persistent SBUF
        et = sing.tile([P, NT, N], bf16)          # E^T   [m-part, mt, n]
        xa = sing.tile([P, NT, CAUG], bf16)       # X|1   [n-part, nt, c]
        cmb = sing.tile([P, NT, BLOC, 2, DIN], bf16)  # [X_b | Y1_b] interleaved
        y2b = sing.tile([P, NT, BLOC, DIN], bf16)
        xgta = sing.tile([P, BLOC, NPAD], bf16)   # [X^T_b ; Y1^T_b] rows 0:64 / 64:128
        xgtb = sing.tile([P, 2, NPAD], bf16)      # Y2^T batch pairs
        wfs = sing.tile([P, 2, DO], bf16)
        nv2r = sing.tile([EMB, N], f32r)
        nv1r = sing.tile([EMB, N], f32r)
        rd2 = sing.tile([P, NT], f32)             # 2/d per node

        ident = sing.tile([P, P], bf16)
        from concourse.masks import make_identity

        make_identity(nc, ident[:, :])

        nc.sync.dma_start(out=nv2r[:, :], in_=nv2[:, :])
        nc.sync.dma_start(out=nv1r[:, :], in_=nv1t[:, :])
        nc.sync.dma_start(out=wfs[:, :, :], in_=wf2[:, :, :].rearrange("c p f -> p c f"))
        for mt in range(NT):
            pm = _tsz(mt)
            nc.sync.dma_start(out=xa[:pm, mt, :], in_=xaug[mt * P : mt * P + pm, :])
            nc.sync.dma_start(
                out=cmb[:pm, mt, :, 0, :], in_=xaug[mt * P : mt * P + pm, :CFREE]
            )

        # ---- fused phase A+B: adjacency + hop1, software-pipelined ----
        # chunk j's z-matmuls interleave 1:4 with chunk j-1's hop1 matmuls so
        # the PE never idles on the ACT exp pipeline (except the initial fill).
        def z_chunk_piece(j, mt):
            n0 = j * 512
            w = min(512, N - n0)
            pm = _tsz(mt)
            pz = pz_.tile([P, 512], f32, tag="pz")
            nc.tensor.matmul(
                pz[:pm, :w],
                lhsT=nv2r[:, mt * P : mt * P + pm],
                rhs=nv1r[:, n0 : n0 + w],
                start=True,
                stop=True,
            )
            nc.scalar.activation(et[:pm, mt, n0 : n0 + w], pz[:pm, :w], AF.Exp)
            nc.vector.tensor_scalar_max(
                et[:pm, mt, n0 : n0 + w], et[:pm, mt, n0 : n0 + w], 1.0
            )

        def hop1_tile(nt):
            pn = _tsz(nt)
            nsl = slice(nt * P, nt * P + pn)
            pu = pu_.tile([P, CAUG], f32, tag="pu")
            for mt in range(NT):
                pm = _tsz(mt)
                nc.tensor.matmul(
                    pu[:pn, :],
                    lhsT=et[:pm, mt, nsl],
                    rhs=xa[:pm, mt, :],
                    start=(mt == 0),
                    stop=(mt == NT - 1),
                )
                yield
            rd = wrk.tile([P, 1], f32, tag="rd")
            nc.vector.reciprocal(rd[:pn, :], pu[:pn, CFREE:CAUG])
            nc.vector.tensor_scalar_mul(
                cmb[:pn, nt, :, 1, :], pu[:pn, :CFREE], rd[:pn, :]
            )
            nc.vector.tensor_scalar_mul(rd2[:pn, nt : nt + 1], rd[:pn, :], 2.0)

        for mt in range(NT):
            z_chunk_piece(0, mt)
        for j in range(4):
            gens = [hop1_tile(nt) for nt in range(4 * j, 4 * j + 4)]
            q = 0
            for g in gens:
                for _ in g:
                    if q % 4 == 0 and j < 3:
                        z_chunk_piece(j + 1, q // 4)
                    q += 1

        # ---- fused phase C+D per node tile: hop2, transposes, combine ----
        unit = 0
        for nt in range(NT):
            pn = _tsz(nt)
            nsl = slice(nt * P, nt * P + pn)
            # hop2: U2 = E @ Y1; Y2 = 2*U2/d - X
            pu2 = pu_.tile([P, CFREE], f32, tag="pu")
            for mt in range(NT):
                pm = _tsz(mt)
                nc.tensor.matmul(
                    pu2[:pn, :],
                    lhsT=et[:pm, mt, nsl],
                    rhs=cmb[:pm, mt, :, 1, :],
                    start=(mt == 0),
                    stop=(mt == NT - 1),
                )
            nc.vector.scalar_tensor_tensor(
                out=y2b[:pn, nt, :, :],
                in0=pu2[:pn, :],
                scalar=rd2[:pn, nt : nt + 1],
                in1=xa[:pn, nt, :CFREE],
                op0=OP.mult,
                op1=OP.subtract,
            )
            # transposes ([pn,128] sources), paired drains
            for h in range(BLOC // 2):
                pt = pz_.tile([P, 2, P], bf16, tag="pz")
                nc.tensor.transpose(
                    pt[:, 0, :pn], cmb[:pn, nt, 2 * h, :, :], ident[:pn, :pn]
                )
                nc.tensor.transpose(
                    pt[:, 1, :pn], cmb[:pn, nt, 2 * h + 1, :, :], ident[:pn, :pn]
                )
                nc.vector.tensor_copy(
                    xgta[:, 2 * h : 2 * h + 2, nsl], pt[:, :, :pn]
                )
            pt = pz_.tile([P, 2, P], bf16, tag="pz")
            for h in range(BLOC // 2):
                nc.tensor.transpose(
                    pt[:, h, :pn], y2b[:pn, nt, 2 * h : 2 * h + 2, :], ident[:pn, :pn]
                )
            nc.vector.tensor_copy(xgtb[:, :, nsl], pt[:, :, :pn])
            if nt > 0:
                cnt = nt - 1
                pn = _tsz(cnt)
                nsl = slice(cnt * P, cnt * P + pn)
                # combine: Z = x_g^T W (per batch), weight by emb, reduce over d
                ee = wrk2.tile([P, DO], bf16, tag="ee")
                nc.sync.dma_start(out=ee[:pn, :], in_=eexp[cnt * P : cnt * P + pn, :, :])
                bs4 = wrk2.tile([P, BLOC, DOUT], f32, tag="bs")
                _bsl = biasf[cnt * P : cnt * P + pn, :]
                nc.sync.dma_start(
        out=bs4[:pn, :, :],
        in_=bass.AP(tensor=_bsl.tensor, offset=_bsl.offset, ap=[_bsl.ap[0], [0, BLOC], _bsl.ap[1]]),
                )
                ob = wrk2.tile([P, BLOC, DOUT], f32, tag="ob")
                ro4 = wrk2.tile([P, BLOC * DOUT], bf16, tag="ro4")
                PATH = "VGAGGAGAVGAGGAGA"
                for b in range(BLOC):
                    pZ = pZ_.tile([P, DO], f32, tag="pZ")
                    for half in range(2):
                        fsl = slice(half * 512, half * 512 + 512)
                        nc.tensor.matmul(
                            pZ[:pn, fsl],
                            lhsT=xgta[:, b, nsl],
                            rhs=wfs[:, 0, fsl],
                            start=True,
                            stop=False,
                        )
                        p0 = (b % 2) * DIN
                        nc.tensor.matmul(
                            pZ[:pn, fsl],
                            lhsT=xgtb[p0 : p0 + DIN, b // 2, nsl],
                            rhs=wfs[p0 : p0 + DIN, 1, fsl],
                            start=False,
                            stop=True,
                        )
                    path = PATH[unit % 16]
                    unit += 1
                    ze = wrk.tile([P, DO], bf16, tag="ze")
                    ro = ro4[:, b * DOUT : (b + 1) * DOUT]
                    if path == "V":
                        nc.vector.tensor_tensor(ze[:pn, :], pZ[:pn, :], ee[:pn, :], OP.mult)
                        zv8 = ze[:pn, :].rearrange("p (o d) -> p o d", d=EMB)
                        v8 = wrk.tile([P, DOUT, 8], bf16, tag="v8")
                        nc.vector.tensor_tensor(v8[:pn], zv8[:, :, 0:8], zv8[:, :, 8:16], OP.add)
                        v4 = wrk.tile([P, DOUT, 4], bf16, tag="v4")
                        nc.vector.tensor_tensor(v4[:pn], v8[:pn, :, 0:4], v8[:pn, :, 4:8], OP.add)
                        with nc.allow_low_precision(reason="16-term bf16 reduce"):
                            nc.vector.tensor_reduce(
                                ro[:pn, :], v4[:pn], axis=mybir.AxisListType.X, op=OP.add
                            )
                    elif path == "A":
                        zs = wrk.tile([P, DO], bf16, tag="zs")
                        nc.scalar.activation(zs[:pn, :], pZ[:pn, :], AF.Copy)
                        nc.vector.tensor_tensor(ze[:pn, :], zs[:pn, :], ee[:pn, :], OP.mult)
                        zv8 = ze[:pn, :].rearrange("p (o d) -> p o d", d=EMB)
                        v8 = wrk.tile([P, DOUT, 8], bf16, tag="v8")
                        nc.vector.tensor_tensor(v8[:pn], zv8[:, :, 0:8], zv8[:, :, 8:16], OP.add)
                        v4 = wrk.tile([P, DOUT, 4], bf16, tag="v4")
                        nc.vector.tensor_tensor(v4[:pn], v8[:pn, :, 0:4], v8[:pn, :, 4:8], OP.add)
                        with nc.allow_low_precision(reason="16-term bf16 reduce"):
                            nc.vector.tensor_reduce(
                                ro[:pn, :], v4[:pn], axis=mybir.AxisListType.X, op=OP.add
                            )
                    else:  # G: ACT drain, DVE mul, GPSIMD tree-reduce + bias add
                        zs = wrk.tile([P, DO], bf16, tag="zs")
                        nc.scalar.activation(zs[:pn, :], pZ[:pn, :], AF.Copy)
                        nc.vector.tensor_tensor(ze[:pn, :], zs[:pn, :], ee[:pn, :], OP.mult)
                        zv = ze[:pn, :].rearrange("p (o d) -> p o d", d=EMB)
                        t8 = wrk.tile([P, DOUT, 8], bf16, tag="t8")
                        nc.gpsimd.tensor_tensor(
                            t8[:pn], zv[:, :, 0:8], zv[:, :, 8:16], OP.add
                        )
                        t4 = wrk.tile([P, DOUT, 4], bf16, tag="t4")
                        nc.gpsimd.tensor_tensor(
                            t4[:pn], t8[:pn, :, 0:4], t8[:pn, :, 4:8], OP.add
                        )
                        t2 = wrk.tile([P, DOUT, 2], bf16, tag="t2")
                        nc.gpsimd.tensor_tensor(
                            t2[:pn], t4[:pn, :, 0:2], t4[:pn, :, 2:4], OP.add
                        )
                        with nc.allow_low_precision(reason="16-term bf16 reduce"):
                            nc.gpsimd.tensor_tensor(
                                ro[:pn].rearrange("p (o u) -> p o u", u=1),
                                t2[:pn, :, 0:1],
                                t2[:pn, :, 1:2],
                                OP.add,
                            )
                nc.vector.tensor_tensor(ob[:pn, :, :], ro4[:pn, :].rearrange("p (b o) -> p b o", b=BLOC), bs4[:pn, :, :], OP.add)
                nc.sync.dma_start(
                    out=outp[:, cnt * P : cnt * P + pn, :].rearrange("b n o -> n b o"),
                    in_=ob[:pn, :, :],
                )


        for cnt in [NT - 1]:
            pn = _tsz(cnt)
            nsl = slice(cnt * P, cnt * P + pn)
            # combine: Z = x_g^T W (per batch), weight by emb, reduce over d
            ee = wrk2.tile([P, DO], bf16, tag="ee")
            nc.sync.dma_start(out=ee[:pn, :], in_=eexp[cnt * P : cnt * P + pn, :, :])
            bs4 = wrk2.tile([P, BLOC, DOUT], f32, tag="bs")
            _bsl = biasf[cnt * P : cnt * P + pn, :]
            nc.sync.dma_start(
        out=bs4[:pn, :, :],
        in_=bass.AP(tensor=_bsl.tensor, offset=_bsl.offset, ap=[_bsl.ap[0], [0, BLOC], _bsl.ap[1]]),
            )
            ob = wrk2.tile([P, BLOC, DOUT], f32, tag="ob")
            ro4 = wrk2.tile([P, BLOC * DOUT], bf16, tag="ro4")
            PATH = "VGAGGAGAVGAGGAGA"
            for b in range(BLOC):
                pZ = pZ_.tile([P, DO], f32, tag="pZ")
                for half in range(2):
                    fsl = slice(half * 512, half * 512 + 512)
                    nc.tensor.matmul(
                        pZ[:pn, fsl],
                        lhsT=xgta[:, b, nsl],
                        rhs=wfs[:, 0, fsl],
                        start=True,
                        stop=False,
                    )
                    p0 = (b % 2) * DIN
                    nc.tensor.matmul(
                        pZ[:pn, fsl],
                        lhsT=xgtb[p0 : p0 + DIN, b // 2, nsl],
                        rhs=wfs[p0 : p0 + DIN, 1, fsl],
                        start=False,
                        stop=True,
                    )
                path = PATH[unit % 16]
                unit += 1
                ze = wrk.tile([P, DO], bf16, tag="ze")
                ro = ro4[:, b * DOUT : (b + 1) * DOUT]
                if path == "V":
                    nc.vector.tensor_tensor(ze[:pn, :], pZ[:pn, :], ee[:pn, :], OP.mult)
                    zv8 = ze[:pn, :].rearrange("p (o d) -> p o d", d=EMB)
                    v8 = wrk.tile([P, DOUT, 8], bf16, tag="v8")
                    nc.vector.tensor_tensor(v8[:pn], zv8[:, :, 0:8], zv8[:, :, 8:16], OP.add)
                    v4 = wrk.tile([P, DOUT, 4], bf16, tag="v4")
                    nc.vector.tensor_tensor(v4[:pn], v8[:pn, :, 0:4], v8[:pn, :, 4:8], OP.add)
                    with nc.allow_low_precision(reason="16-term bf16 reduce"):
                        nc.vector.tensor_reduce(
                            ro[:pn, :], v4[:pn], axis=mybir.AxisListType.X, op=OP.add
                        )
                elif path == "A":
                    zs = wrk.tile([P, DO], bf16, tag="zs")
                    nc.scalar.activation(zs[:pn, :], pZ[:pn, :], AF.Copy)
                    nc.vector.tensor_tensor(ze[:pn, :], zs[:pn, :], ee[:pn, :], OP.mult)
                    zv8 = ze[:pn, :].rearrange("p (o d) -> p o d", d=EMB)
                    v8 = wrk.tile([P, DOUT, 8], bf16, tag="v8")
                    nc.vector.tensor_tensor(v8[:pn], zv8[:, :, 0:8], zv8[:, :, 8:16], OP.add)
                    v4 = wrk.tile([P, DOUT, 4], bf16, tag="v4")
                    nc.vector.tensor_tensor(v4[:pn], v8[:pn, :, 0:4], v8[:pn, :, 4:8], OP.add)
                    with nc.allow_low_precision(reason="16-term bf16 reduce"):
                        nc.vector.tensor_reduce(
                            ro[:pn, :], v4[:pn], axis=mybir.AxisListType.X, op=OP.add
                        )
                else:  # G: ACT drain, DVE mul, GPSIMD tree-reduce + bias add
                    zs = wrk.tile([P, DO], bf16, tag="zs")
                    nc.scalar.activation(zs[:pn, :], pZ[:pn, :], AF.Copy)
                    nc.vector.tensor_tensor(ze[:pn, :], zs[:pn, :], ee[:pn, :], OP.mult)
                    zv = ze[:pn, :].rearrange("p (o d) -> p o d", d=EMB)
                    t8 = wrk.tile([P, DOUT, 8], bf16, tag="t8")
                    nc.gpsimd.tensor_tensor(
                        t8[:pn], zv[:, :, 0:8], zv[:, :, 8:16], OP.add
                    )
                    t4 = wrk.tile([P, DOUT, 4], bf16, tag="t4")
                    nc.gpsimd.tensor_tensor(
                        t4[:pn], t8[:pn, :, 0:4], t8[:pn, :, 4:8], OP.add
                    )
                    t2 = wrk.tile([P, DOUT, 2], bf16, tag="t2")
                    nc.gpsimd.tensor_tensor(
                        t2[:pn], t4[:pn, :, 0:2], t4[:pn, :, 2:4], OP.add
                    )
                    with nc.allow_low_precision(reason="16-term bf16 reduce"):
                        nc.gpsimd.tensor_tensor(
                            ro[:pn].rearrange("p (o u) -> p o u", u=1),
                            t2[:pn, :, 0:1],
                            t2[:pn, :, 1:2],
                            OP.add,
                        )
            nc.vector.tensor_tensor(ob[:pn, :, :], ro4[:pn, :].rearrange("p (b o) -> p b o", b=BLOC), bs4[:pn, :, :], OP.add)
            nc.sync.dma_start(
                out=outp[:, cnt * P : cnt * P + pn, :].rearrange("b n o -> n b o"),
                in_=ob[:pn, :, :],
            )


    nc.compile()
    return nc


_NC_CACHE: list = []


def _get_nc():
    if not _NC_CACHE:
        _NC_CACHE.append(_build())
    return _NC_CACHE[0]


def _prep_shared(node_embeddings, nodevec1, nodevec2, weights_pool, bias_pool):
    nv2 = np.ascontiguousarray(nodevec2, dtype=np.float32)
    nv1t = np.ascontiguousarray(np.asarray(nodevec1, np.float32).T)
    wf = (
        np.transpose(np.asarray(weights_pool, np.float32), (1, 2, 3, 0))
        .reshape(KI, DO)
    )  # rows (k,i), cols (o,d) -- d innermost for contiguous reduce
    wf2 = np.zeros((2, P, DO), np.float32)
    wf2[0] = wf[0:P]
    wf2[1, 0:DIN] = wf[P:KI]
    wf2[1, DIN:P] = wf[P:KI]  # k2 chunk replicated so odd-batch lhsT base matches
    emb = np.asarray(node_embeddings, np.float32)
    eexp = np.ascontiguousarray(
        np.broadcast_to(emb[:, None, :], (N, DOUT, EMB))
    ).astype(BF16)
    biasf = (emb @ np.asarray(bias_pool, np.float32)).astype(np.float32)
    return {
        "nv2": nv2,
        "nv1t": nv1t,
        "wf2": wf2.astype(BF16),
        "eexp": eexp,
        "biasf": biasf,
    }


def _prep_core(x, core):
    xl = np.asarray(x[core * BLOC : (core + 1) * BLOC], np.float32)  # [4, N, 64]
    xn = np.ascontiguousarray(xl.transpose(1, 0, 2).reshape(N, CFREE))
    xaug = np.empty((N, CAUG), np.float32)
    xaug[:, :CFREE] = xn
    xaug[:, CFREE] = 1.0
    return {"xaug": xaug.astype(BF16)}


def run(x, node_embeddings, nodevec1, nodevec2, weights_pool, bias_pool, **spmd_kwargs):
    nc = _get_nc()
    shared = _prep_shared(node_embeddings, nodevec1, nodevec2, weights_pool, bias_pool)
    in_maps = [{**shared, **_prep_core(x, c)} for c in range(CORES)]
    res = run_bass_kernel_spmd(nc, in_maps, core_ids=list(range(CORES)), **spmd_kwargs)
    out = np.concatenate([res.results[c]["out"] for c in range(CORES)], axis=0)
    return np.ascontiguousarray(out.astype(np.float32)), res


def kernel(x, node_embeddings, nodevec1, nodevec2, weights_pool, bias_pool):
    out, _ = run(x, node_embeddings, nodevec1, nodevec2, weights_pool, bias_pool)
    return out

